# revision 1
# baseline (speedup 1.0000x reference)
"""GCN encoder (2x GCNConv + global mean pool) on 8 Trainium2 NeuronCores.

Sharding: nodes are dst-sharded across 8 cores (12500 nodes each, padded to
12544).  Each core computes h_pre = x_c @ W for its slice, the slices are
AllGathered, then each core gathers h_pre[src] for its incident edges
(dma_gather, int16 window-local indices), scales by edge weight on DVE, and
CCE-scatter-adds into its local aggregation buffer (dma_scatter_add).
Bias+relu (+ next-layer matmul) run per 128-node tile; global mean pool is a
one-hot matmul accumulated in PSUM with host-precomputed 1/count weights,
finished with a tiny AllReduce so every core holds the full [64,64] output.
"""
import sys
sys.path.insert(0, "/opt/trn_rl_repo")

import numpy as np

P = 128
CHUNK = 4096          # edges per gather/scatter instruction
DMA_SCRATCH = 131072  # SWDGE descriptor carveout bytes/partition (8192 descs)


# ---------------------------------------------------------------- host prep

def _wrap16(a):
    return a.reshape(-1, 16).T.copy()


def _wrap128(a):
    return a.reshape(-1, P).T.copy()


def prep_host(x, edge_index, edge_weight, batch, num_graphs, chunk=CHUNK):
    """Shard + pack inputs. Returns (in_maps, meta)."""
    x = np.ascontiguousarray(np.asarray(x, dtype=np.float32))
    src = np.asarray(edge_index[0], dtype=np.int64)
    dst = np.asarray(edge_index[1], dtype=np.int64)
    ew = np.asarray(edge_weight, dtype=np.float32)
    batch = np.asarray(batch, dtype=np.int64)
    G = int(num_graphs)

    n_cores = 8
    N, IN_CH = x.shape
    npc = N // n_cores                      # real nodes per core
    # padded nodes per core; always leaves >=1 spare row (scatter pad target)
    npp = ((npc + P) // P) * P
    n_tiles = npp // P
    cpw = max(1, 32768 // npp)              # cores per gather window
    n_win = (n_cores + cpw - 1) // cpw
    win = cpw * npp                         # padded rows per window

    E = src.shape[0]
    core_of = dst // npc
    psrc = (src // npc) * npp + (src % npc)     # src id in padded node space
    win_of = psrc // win
    gidx = (psrc % win).astype(np.int16)
    sidx = (dst % npc).astype(np.int16)

    # rank of each edge within its (core, window, dst) group; scatter
    # instructions are sliced so every instruction sees each dst at most once
    # (the CCE read-modify-write races on duplicate rows within one
    # instruction).
    key = (core_of * n_win + win_of) * N + dst
    o0 = np.argsort(key, kind="stable")
    ks = key[o0]
    new_grp = np.r_[True, ks[1:] != ks[:-1]]
    grp_start = np.maximum.accumulate(np.where(new_grp, np.arange(E), 0))
    rank = np.empty(E, np.int64)
    rank[o0] = np.arange(E) - grp_start

    order = np.lexsort((dst, rank, win_of, core_of))
    core_s = core_of[order]
    win_s = win_of[order]
    rank_s = rank[order]
    gidx_s, sidx_s, ew_s = gidx[order], sidx[order], ew[order]

    # run = (window, rank) group; common (max-over-cores, 128-padded) lengths
    max_rank = int(rank.max()) + 1
    counts = np.zeros((n_cores, n_win, max_rank), np.int64)
    np.add.at(counts, (core_s, win_s, rank_s), 1)
    run_len = np.zeros((n_win, max_rank), np.int64)
    for w in range(n_win):
        for r in range(max_rank):
            m = counts[:, w, r].max()
            run_len[w, r] = ((m + P - 1) // P) * P

    # slices (shared schedule): each slice lies inside one run -> unique dsts
    slices = []   # (window, stream_start, length)
    run_off = {}  # (w, r) -> stream offset
    off = 0
    for w in range(n_win):
        for r in range(max_rank):
            L = int(run_len[w, r])
            if L == 0:
                continue
            run_off[(w, r)] = off
            p = 0
            while p < L:
                ln = min(chunk, L - p)
                slices.append((w, off + p, ln))
                p += ln
            off += L
    tot = off

    cnt = np.bincount(batch, minlength=G).astype(np.float32)
    inv_cnt = 1.0 / np.maximum(cnt, 1.0)

    in_maps = []
    # per-edge stream position: run_off + within-run index. Edges are sorted
    # by (core, window, rank, dst); within-run index is the running index
    # inside each (core, window, rank) group.
    gk = (core_s * n_win + win_s) * max_rank + rank_s
    new_g = np.r_[True, gk[1:] != gk[:-1]]
    gstart = np.maximum.accumulate(np.where(new_g, np.arange(E), 0))
    within = np.arange(E) - gstart

    for c in range(n_cores):
        g_pad = np.zeros(tot, np.int16)
        s_pad = np.full(tot, npc, np.int16)   # pads: +0 into unused row npc
        e_pad = np.zeros(tot, np.float32)
        sel = core_s == c
        pos = np.array([run_off[(w, r)] for w, r in
                        zip(win_s[sel], rank_s[sel])]) + within[sel]
        g_pad[pos] = gidx_s[sel]
        s_pad[pos] = sidx_s[sel]
        e_pad[pos] = ew_s[sel]

        xT = np.zeros((IN_CH, npp), np.float32)
        xT[:, :npc] = x[c * npc:(c + 1) * npc].T
        bc = batch[c * npc:(c + 1) * npc]
        pool = np.zeros((npp, G), np.float32)
        pool[np.arange(npc), bc] = inv_cnt[bc]

        in_maps.append({
            "xT": xT,
            "gidx": np.tile(_wrap16(g_pad), (8, 1)),
            "sidx": np.tile(_wrap16(s_pad), (8, 1)),
            "ew": _wrap128(e_pad),
            "pool": pool,
        })

    meta = dict(n_cores=n_cores, N=N, IN_CH=IN_CH, npc=npc, npp=npp,
                n_tiles=n_tiles, n_win=n_win, win=win, slices=slices,
                tot=tot, G=G)
    return in_maps, meta


# ------------------------------------------------------------ program build

def build_program(meta, W1, b1, W2, b2, queues=1, inplace=False,
                  gm_bufs=2, scratch=DMA_SCRATCH, gq=None, sq=None):
    import concourse.bass as bass
    import concourse.bacc as bacc
    import concourse.tile as tile
    import concourse.mybir as mybir
    from concourse.masks import make_identity

    f32 = mybir.dt.float32
    i16 = mybir.dt.int16
    NCORES = meta["n_cores"]
    IN_CH, npp = meta["IN_CH"], meta["npp"]
    NT, NW, WIN = meta["n_tiles"], meta["n_win"], meta["win"]
    SLICES, TOT, G = meta["slices"], meta["tot"], meta["G"]
    HID = W1.shape[1]

    if gq is None:
        gq = (0,)
    if sq is None:
        sq = (queues - 1,)
    n_queues = max(max(gq), max(sq)) + 1
    nc = bacc.Bacc("TRN2", target_bir_lowering=False, debug=False,
                   num_devices=NCORES, dynamic_dma_scratch_size=scratch,
                   num_swdge_queues=n_queues)

    xT_d = nc.dram_tensor("xT", [IN_CH, npp], f32, kind="ExternalInput")
    gidx_d = nc.dram_tensor("gidx", [P, TOT // 16], i16, kind="ExternalInput")
    sidx_d = nc.dram_tensor("sidx", [P, TOT // 16], i16, kind="ExternalInput")
    ew_d = nc.dram_tensor("ew", [P, TOT // P], f32, kind="ExternalInput")
    pool_d = nc.dram_tensor("pool", [npp, G], f32, kind="ExternalInput")
    W1_d = nc.dram_tensor("W1", [IN_CH, HID], f32, kind="ExternalInput")
    b1_d = nc.dram_tensor("b1", [HID, 1], f32, kind="ExternalInput")
    W2_d = nc.dram_tensor("W2", [HID, HID], f32, kind="ExternalInput")
    b2_d = nc.dram_tensor("b2", [P, HID], f32, kind="ExternalInput")
    out_d = nc.dram_tensor("out", [HID, G], f32, kind="ExternalOutput")

    with tile.TileContext(nc) as tc:
        with (
            tc.tile_pool(name="const", bufs=1) as const,
            tc.tile_pool(name="dram", bufs=1, space="DRAM") as dram,
            tc.tile_pool(name="io", bufs=3) as io,
            tc.tile_pool(name="gm", bufs=gm_bufs) as gm,
            tc.tile_pool(name="hsb", bufs=3) as hsb,
            tc.tile_pool(name="psA", bufs=2, space="PSUM") as psA,
            tc.tile_pool(name="psT", bufs=2, space="PSUM") as psT,
            tc.tile_pool(name="psB", bufs=2, space="PSUM") as psB,
            tc.tile_pool(name="psPool", bufs=1, space="PSUM") as psPool,
        ):
            W1_s = const.tile([IN_CH, HID], f32)
            nc.sync.dma_start(W1_s[:], W1_d[:])
            W2_s = const.tile([HID, HID], f32)
            nc.sync.dma_start(W2_s[:], W2_d[:])
            b1_s = const.tile([HID, 1], f32)
            nc.sync.dma_start(b1_s[:], b1_d[:])
            b2_s = const.tile([P, HID], f32)
            nc.sync.dma_start(b2_s[:], b2_d[:])
            ident = const.tile([P, P], f32)
            make_identity(nc, ident[:])

            hpre_loc = dram.tile([npp, HID], f32)
            hpre_full = dram.tile([NCORES * npp, HID], f32)
            h2pre_loc = dram.tile([npp, HID], f32)
            h2pre_full = dram.tile([NCORES * npp, HID], f32)
            agg1_d = dram.tile([npp, HID], f32)
            agg2_d = dram.tile([npp, HID], f32)

            # zero the scatter accumulators on device (CCE adds into them)
            zt = const.tile([P, HID], f32)
            nc.gpsimd.memset(zt[:], 0.0)
            for t in range(NT):
                nc.sync.dma_start(agg1_d[t * P:(t + 1) * P, :], zt[:])
                nc.sync.dma_start(agg2_d[t * P:(t + 1) * P, :], zt[:])

            # ---- phase A: h1_pre = x @ W1 (node-major via xT as lhsT)
            for t in range(NT):
                xs = io.tile([IN_CH, P], f32, tag="xs")
                nc.sync.dma_start(xs[:], xT_d[:, t * P:(t + 1) * P])
                ps = psA.tile([P, HID], f32, tag="psA")
                nc.tensor.matmul(ps[:], lhsT=xs[:], rhs=W1_s[:],
                                 start=True, stop=True)
                hb = hsb.tile([P, HID], f32, tag="hb")
                nc.vector.tensor_copy(hb[:], ps[:])
                nc.sync.dma_start(hpre_loc[t * P:(t + 1) * P, :], hb[:])

            nc.gpsimd.collective_compute(
                "AllGather", mybir.AluOpType.bypass,
                replica_groups=[list(range(NCORES))],
                ins=[hpre_loc.opt()], outs=[hpre_full.opt()])

            # ---- phase C/F: gather -> scale -> scatter-add
            def message_pass(h_full, agg_dst):
                for i, (w, start, ln) in enumerate(SLICES):
                    src_hi = min((w + 1) * WIN, NCORES * npp)
                    src_win = h_full[w * WIN:src_hi, :]
                    s16, c = ln // 16, ln // P
                    o16, oc = start // 16, start // P
                    git = io.tile([P, s16], i16, tag="git")
                    sit = io.tile([P, s16], i16, tag="sit")
                    ewt = io.tile([P, c, 1], f32, tag="ewt")
                    nc.sync.dma_start(git[:], gidx_d[:, o16:o16 + s16])
                    nc.sync.dma_start(sit[:], sidx_d[:, o16:o16 + s16])
                    nc.sync.dma_start(ewt[:, :, 0], ew_d[:, oc:oc + c])
                    g = gm.tile([P, c, HID], f32, tag="g")
                    nc.gpsimd.dma_gather(
                        out_ap=g[:], in_ap=src_win, idxs_ap=git[:],
                        num_idxs=ln, num_idxs_reg=ln, elem_size=HID,
                        single_packet=False, queue_num=gq[i % len(gq)])
                    if inplace:
                        m = g
                    else:
                        m = gm.tile([P, c, HID], f32, tag="m")
                    nc.vector.tensor_tensor(
                        out=m[:], in0=g[:],
                        in1=ewt[:].to_broadcast([P, c, HID]),
                        op=mybir.AluOpType.mult)
                    nc.gpsimd.dma_scatter_add(
                        agg_dst[:], m[:], sit[:], ln, ln, HID,
                        single_packet=False, queue_num=sq[i % len(sq)])

            message_pass(hpre_full, agg1_d)

            # ---- phase D: h1 = relu(agg1+b1); h2_pre = h1 @ W2
            for t in range(NT):
                a = hsb.tile([P, HID], f32, tag="a")
                nc.sync.dma_start(a[:], agg1_d[t * P:(t + 1) * P, :])
                tp = psT.tile([HID, P], f32, tag="tp")
                nc.tensor.transpose(out=tp[:], in_=a[:], identity=ident[:])
                h1t = hsb.tile([HID, P], f32, tag="h1t")
                nc.scalar.activation(h1t[:], tp[:],
                                     mybir.ActivationFunctionType.Relu,
                                     bias=b1_s[:], scale=1.0)
                ps2 = psB.tile([P, HID], f32, tag="psB")
                nc.tensor.matmul(ps2[:], lhsT=h1t[:], rhs=W2_s[:],
                                 start=True, stop=True)
                hb2 = hsb.tile([P, HID], f32, tag="hb2")
                nc.vector.tensor_copy(hb2[:], ps2[:])
                nc.sync.dma_start(h2pre_loc[t * P:(t + 1) * P, :], hb2[:])

            nc.gpsimd.collective_compute(
                "AllGather", mybir.AluOpType.bypass,
                replica_groups=[list(range(NCORES))],
                ins=[h2pre_loc.opt()], outs=[h2pre_full.opt()])

            message_pass(h2pre_full, agg2_d)

            # ---- phase G: h2 = relu(agg2+b2); pooled += h2.T @ pool_onehot
            pool_ps = psPool.tile([HID, G], f32)
            for t in range(NT):
                a2 = hsb.tile([P, HID], f32, tag="a2")
                nc.sync.dma_start(a2[:], agg2_d[t * P:(t + 1) * P, :])
                h2 = hsb.tile([P, HID], f32, tag="h2")
                nc.vector.tensor_tensor(out=h2[:], in0=a2[:], in1=b2_s[:],
                                        op=mybir.AluOpType.add)
                nc.vector.tensor_scalar(out=h2[:], in0=h2[:], scalar1=0.0,
                                        scalar2=None, op0=mybir.AluOpType.max)
                pt = hsb.tile([P, G], f32, tag="pt")
                nc.sync.dma_start(pt[:], pool_d[t * P:(t + 1) * P, :])
                nc.tensor.matmul(pool_ps[:], lhsT=h2[:], rhs=pt[:],
                                 start=(t == 0), stop=(t == NT - 1))

            pool_sb = const.tile([HID, G], f32)
            nc.vector.tensor_copy(pool_sb[:], pool_ps[:])
            pool_in = dram.tile([HID, G], f32)
            pool_out = dram.tile([HID, G], f32)
            nc.sync.dma_start(pool_in[:], pool_sb[:])
            nc.gpsimd.collective_compute(
                "AllReduce", mybir.AluOpType.add,
                replica_groups=[list(range(NCORES))],
                ins=[pool_in.opt()], outs=[pool_out.opt()])
            nc.sync.dma_start(out_d[:], pool_out[:])

    nc.compile()
    return nc


# ------------------------------------------------------------------ runner

def run_spmd(nc, in_maps, n_cores=8, time_runs=0):
    """run_bass_via_pjrt clone that can re-execute the jitted NEFF for timing.

    Returns (results, exec_seconds|None): results from the first execution,
    exec_seconds = best wall-clock of `time_runs` repeat executions with
    device-resident inputs.
    """
    import time as _time
    import jax
    import jax.numpy as jnp
    from jax.sharding import Mesh, PartitionSpec
    from jax.experimental.shard_map import shard_map
    from concourse import bass2jax
    import concourse.mybir as mybir

    bass2jax.install_neuronx_cc_hook()
    part_name = nc.partition_id_tensor.name if nc.partition_id_tensor else None
    in_names, out_names, out_avals, zero_shapes = [], [], [], []
    for alloc in nc.m.functions[0].allocations:
        if not isinstance(alloc, mybir.MemoryLocationSet):
            continue
        name = alloc.memorylocations[0].name
        if alloc.kind == "ExternalInput":
            if name != part_name:
                in_names.append(name)
        elif alloc.kind == "ExternalOutput":
            out_names.append(name)
            shape = tuple(alloc.tensor_shape)
            dtype = mybir.dt.np(alloc.dtype)
            out_avals.append(jax.core.ShapedArray(shape, dtype))
            zero_shapes.append((shape, dtype))
    n_params = len(in_names)
    all_names = in_names + out_names
    if part_name is not None:
        all_names = all_names + [part_name]

    def _body(*args):
        operands = list(args)
        if part_name is not None:
            operands.append(bass2jax.partition_id_tensor())
        return tuple(bass2jax._bass_exec_p.bind(
            *operands, out_avals=tuple(out_avals), in_names=tuple(all_names),
            out_names=tuple(out_names), lowering_input_output_aliases=(),
            sim_require_finite=True, sim_require_nnan=True, nc=nc))

    devices = jax.devices()[:n_cores]
    mesh = Mesh(np.asarray(devices), ("core",))
    donate = tuple(range(n_params, n_params + len(out_names)))
    sharded = jax.jit(
        shard_map(_body, mesh=mesh,
                  in_specs=(PartitionSpec("core"),) * (n_params + len(out_names)),
                  out_specs=(PartitionSpec("core"),) * len(out_names),
                  check_rep=False),
        donate_argnums=donate, keep_unused=True)

    concat_in = [np.concatenate([np.asarray(in_maps[c][n]) for c in range(n_cores)], 0)
                 for n in in_names]
    def zeros():
        return [np.zeros((n_cores * s[0], *s[1:]), d) for s, d in zero_shapes]

    out_arrs = sharded(*concat_in, *zeros())
    jax.block_until_ready(out_arrs)
    results = [
        {n: np.asarray(out_arrs[i]).reshape(n_cores, *out_avals[i].shape)[c]
         for i, n in enumerate(out_names)}
        for c in range(n_cores)
    ]

    best = None
    if time_runs:
        dev_in = [jax.device_put(jnp.asarray(a)) for a in concat_in]
        jax.block_until_ready(dev_in)
        for _ in range(time_runs):
            z = zeros()
            t0 = _time.perf_counter()
            o = sharded(*dev_in, *z)
            jax.block_until_ready(o)
            dt = _time.perf_counter() - t0
            best = dt if best is None else min(best, dt)
    return results, best


_LAST_EXEC_S = None


def kernel(x, edge_index, edge_weight, batch, num_graphs, W1, b1, W2, b2):
    global _LAST_EXEC_S
    W1 = np.ascontiguousarray(np.asarray(W1, np.float32))
    b1 = np.ascontiguousarray(np.asarray(b1, np.float32)).reshape(-1, 1)
    W2 = np.ascontiguousarray(np.asarray(W2, np.float32))
    b2v = np.ascontiguousarray(np.asarray(b2, np.float32)).reshape(1, -1)
    b2r = np.tile(b2v, (P, 1))

    in_maps, meta = prep_host(x, edge_index, edge_weight, batch, num_graphs)
    for m in in_maps:
        m["W1"], m["b1"], m["W2"], m["b2"] = W1, b1, W2, b2r

    # gather on SWDGE queue 0, scatter-add on queue 1: overlaps the two
    # custom-DMA legs. 3/4-queue splits measured a few ms faster but caused
    # an intermittent NRT_EXEC_UNIT_UNRECOVERABLE device crash; 2 queues is
    # the fastest configuration that proved stable across processes.
    nc = build_program(meta, W1, b1, W2, b2r, gq=(0,), sq=(1,))
    import os
    time_runs = int(os.environ.get("GCN_TIME_RUNS", "0"))
    results, best = run_spmd(nc, in_maps, meta["n_cores"], time_runs=time_runs)
    _LAST_EXEC_S = best
    return np.ascontiguousarray(results[0]["out"].T.astype(np.float32))



# revision 8
# speedup vs baseline: 1.2747x; 1.2747x over previous
"""GCN encoder (2x GCNConv + global mean pool) on 8 Trainium2 NeuronCores.

Sharding: nodes are dst-sharded across 8 cores (12500 nodes each, padded to
12544).  Each core computes h_pre = x_c @ W for its slice and the slices are
AllGathered.  Message passing is gather-only: edges are grouped by
(src window, dst tile) with each group padded to a multiple of 128;
dma_gather (4 SWDGE queues, round-robin, so descriptor generation runs on
all 4 Q7 core pairs) pulls h_pre[src] in 4096-edge chunks with int16
window-local indices, DVE scales by edge weight, and aggregation runs on
the TensorEngine as one-hot matmuls accumulated in PSUM — a 128-edge
chunk's [128e x 128d] one-hot is built by DVE is_equal of the chunk's
dst-local ids against a host-provided iota row.  Per-tile results accumulate
into an SBUF buffer; there is no dma_scatter_add and no HBM read-modify-
write.  Bias+relu and the next-layer matmul read the SBUF accumulator per
128-node tile; global mean pool is a one-hot matmul with host-precomputed
1/count weights, finished with a tiny AllReduce so every core holds the
full [64,64] output.
"""
import sys
sys.path.insert(0, "/opt/trn_rl_repo")

import numpy as np

P = 128
GCHUNK = 4096         # edges per gather instruction
DMA_SCRATCH = 65536   # SWDGE descriptor carveout bytes/partition


def _wrap16(a):
    return a.reshape(-1, 16).T.copy()


def _wrap128(a):
    return a.reshape(-1, P).T.copy()


# ---------------------------------------------------------------- host prep

def prep_host(x, edge_index, edge_weight, batch, num_graphs, gchunk=GCHUNK):
    """Shard + pack inputs. Returns (in_maps, meta)."""
    x = np.ascontiguousarray(np.asarray(x, dtype=np.float32))
    src = np.asarray(edge_index[0], dtype=np.int64)
    dst = np.asarray(edge_index[1], dtype=np.int64)
    ew = np.asarray(edge_weight, dtype=np.float32)
    batch = np.asarray(batch, dtype=np.int64)
    G = int(num_graphs)

    n_cores = 8
    N, IN_CH = x.shape
    E = src.shape[0]
    npc = N // n_cores                      # real nodes per core
    npp = ((npc + P) // P) * P              # padded nodes per core
    nt = npp // P                           # dst tiles per core
    cpw = max(1, 32768 // npp)              # cores per gather window
    n_win = (n_cores + cpw - 1) // cpw
    win = cpw * npp                         # padded rows per window

    core = dst // npc
    tl = (dst % npc) // P                   # dst tile within core
    dl = ((dst % npc) % P).astype(np.float32)   # dst row within tile
    psrc = (src // npc) * npp + (src % npc)     # src id in padded node space
    w = psrc // win
    g16 = (psrc % win).astype(np.int16)     # window-local src id

    # shared (max-over-cores) 128-padded group lengths for (window, tile)
    nwt = n_win * nt
    key = core * nwt + w * nt + tl
    cnt = np.bincount(key, minlength=n_cores * nwt).reshape(n_cores, n_win, nt)
    cmax = cnt.max(axis=0)
    pl = ((cmax + P - 1) // P) * P
    pl[cmax == 0] = 0                       # [n_win, nt]
    flat = pl.ravel()
    off = np.concatenate([[0], np.cumsum(flat)])[:-1].reshape(n_win, nt)
    TOT = int(flat.sum())

    # per-edge stream position: group offset + within-(core,w,t) rank
    order = np.argsort(key, kind="stable")
    ks = key[order]
    new_g = np.r_[True, ks[1:] != ks[:-1]]
    gstart = np.maximum.accumulate(np.where(new_g, np.arange(E), 0))
    within = np.arange(E) - gstart
    pos = off[w[order], tl[order]] + within
    core_s = core[order]

    cnt_b = np.bincount(batch, minlength=G).astype(np.float32)
    inv_cnt = 1.0 / np.maximum(cnt_b, 1.0)

    g16_s, dl_s, ew_s = g16[order], dl[order], ew[order]
    in_maps = []
    for c in range(n_cores):
        sel = core_s == c
        p = pos[sel]
        gi = np.zeros(TOT, np.int16)          # pads gather row 0
        dli = np.full(TOT, -1.0, np.float32)  # pads match no dst row
        ewi = np.zeros(TOT, np.float32)       # pads contribute 0
        gi[p] = g16_s[sel]
        dli[p] = dl_s[sel]
        ewi[p] = ew_s[sel]

        xT = np.zeros((IN_CH, npp), np.float32)
        xT[:, :npc] = x[c * npc:(c + 1) * npc].T
        bc = batch[c * npc:(c + 1) * npc]
        pool = np.zeros((npp, G), np.float32)
        pool[np.arange(npc), bc] = inv_cnt[bc]

        in_maps.append({
            "xT": xT,
            "iota": np.tile(np.arange(P, dtype=np.float32), (P, 1)),
            "gidx": np.tile(_wrap16(gi), (8, 1)),
            "dloc": _wrap128(dli),
            "ew": _wrap128(ewi),
            "pool": pool,
        })

    # gather slices (w, stream start, len), each within one window
    slices = []
    for w_ in range(n_win):
        s = int(off[w_, 0])
        L = int(pl[w_].sum())
        p0 = 0
        while p0 < L:
            ln = min(gchunk, L - p0)
            slices.append((w_, s + p0, ln))
            p0 += ln

    # aggregation groups (tile, first chunk, n chunks) in stream order
    groups = []
    for w_ in range(n_win):
        for t_ in range(nt):
            L = int(pl[w_, t_])
            if L:
                groups.append((t_, int(off[w_, t_]) // P, L // P))

    meta = dict(n_cores=n_cores, N=N, IN_CH=IN_CH, npc=npc, npp=npp,
                n_tiles=nt, n_win=n_win, win=win, slices=slices,
                groups=groups, tot=TOT, G=G)
    return in_maps, meta


# ------------------------------------------------------------ program build

def build_program(meta, W1, b1, W2, b2, scratch=DMA_SCRATCH, gq=(0, 1, 2, 3),
                  skip_mp=False, skip_ag=False):
    import concourse.bass as bass
    import concourse.bacc as bacc
    import concourse.tile as tile
    import concourse.mybir as mybir
    from concourse.masks import make_identity

    f32 = mybir.dt.float32
    i16 = mybir.dt.int16
    NCORES = meta["n_cores"]
    IN_CH, npp = meta["IN_CH"], meta["npp"]
    NT, NW, WIN = meta["n_tiles"], meta["n_win"], meta["win"]
    SLICES, GROUPS, TOT, G = meta["slices"], meta["groups"], meta["tot"], meta["G"]
    HID = W1.shape[1]

    n_queues = max(gq) + 1
    nc = bacc.Bacc("TRN2", target_bir_lowering=False, debug=False,
                   num_devices=NCORES, dynamic_dma_scratch_size=scratch,
                   num_swdge_queues=n_queues)

    xT_d = nc.dram_tensor("xT", [IN_CH, npp], f32, kind="ExternalInput")
    iota_d = nc.dram_tensor("iota", [P, P], f32, kind="ExternalInput")
    gidx_d = nc.dram_tensor("gidx", [P, TOT // 16], i16, kind="ExternalInput")
    dloc_d = nc.dram_tensor("dloc", [P, TOT // P], f32, kind="ExternalInput")
    ew_d = nc.dram_tensor("ew", [P, TOT // P], f32, kind="ExternalInput")
    pool_d = nc.dram_tensor("pool", [npp, G], f32, kind="ExternalInput")
    W1_d = nc.dram_tensor("W1", [IN_CH, HID], f32, kind="ExternalInput")
    b1_d = nc.dram_tensor("b1", [HID, 1], f32, kind="ExternalInput")
    W2_d = nc.dram_tensor("W2", [HID, HID], f32, kind="ExternalInput")
    b2_d = nc.dram_tensor("b2", [P, HID], f32, kind="ExternalInput")
    out_d = nc.dram_tensor("out", [HID, G], f32, kind="ExternalOutput")

    # chunk index -> slice index
    chunk2slice = []
    slice_chunk0 = []
    for si, (w_, s, ln) in enumerate(SLICES):
        slice_chunk0.append(s // P)
        chunk2slice.extend([si] * (ln // P))

    with tile.TileContext(nc) as tc:
        with (
            tc.tile_pool(name="const", bufs=1) as const,
            tc.tile_pool(name="dram", bufs=1, space="DRAM") as dram,
            tc.tile_pool(name="io", bufs=3) as io,
            tc.tile_pool(name="gm", bufs=3) as gm,
            tc.tile_pool(name="oh", bufs=4) as ohp,
            tc.tile_pool(name="hsb", bufs=3) as hsb,
            tc.tile_pool(name="psA", bufs=2, space="PSUM") as psA,
            tc.tile_pool(name="psAgg", bufs=2, space="PSUM") as psAgg,
            tc.tile_pool(name="psT", bufs=1, space="PSUM") as psT,
            tc.tile_pool(name="psB", bufs=2, space="PSUM") as psB,
            tc.tile_pool(name="psPool", bufs=1, space="PSUM") as psPool,
        ):
            W1_s = const.tile([IN_CH, HID], f32)
            nc.sync.dma_start(W1_s[:], W1_d[:])
            W2_s = const.tile([HID, HID], f32)
            nc.sync.dma_start(W2_s[:], W2_d[:])
            b1_s = const.tile([HID, 1], f32)
            nc.sync.dma_start(b1_s[:], b1_d[:])
            b2_s = const.tile([P, HID], f32)
            nc.sync.dma_start(b2_s[:], b2_d[:])
            ident = const.tile([P, P], f32)
            make_identity(nc, ident[:])
            # iota_row[p, j] = j  (host-provided f32 constant)
            iota_row = const.tile([P, P], f32)
            nc.sync.dma_start(iota_row[:], iota_d[:])

            hpre_loc = dram.tile([npp, HID], f32)
            hpre_full = dram.tile([NCORES * npp, HID], f32)
            h2pre_loc = dram.tile([npp, HID], f32)
            h2pre_full = dram.tile([NCORES * npp, HID], f32)

            agg1_sb = const.tile([P, NT * HID], f32)
            nc.gpsimd.memset(agg1_sb[:], 0.0)
            agg2_sb = const.tile([P, NT * HID], f32)
            nc.gpsimd.memset(agg2_sb[:], 0.0)

            # ---- phase A: h1_pre = x @ W1 (node-major via xT as lhsT)
            for t in range(NT):
                xs = io.tile([IN_CH, P], f32, tag="xs")
                nc.sync.dma_start(xs[:], xT_d[:, t * P:(t + 1) * P])
                ps = psA.tile([P, HID], f32, tag="psA")
                nc.tensor.matmul(ps[:], lhsT=xs[:], rhs=W1_s[:],
                                 start=True, stop=True)
                hb = hsb.tile([P, HID], f32, tag="hb")
                nc.vector.tensor_copy(hb[:], ps[:])
                nc.sync.dma_start(hpre_loc[t * P:(t + 1) * P, :], hb[:])

            if not skip_ag:
                nc.gpsimd.collective_compute(
                    "AllGather", mybir.AluOpType.bypass,
                    replica_groups=[list(range(NCORES))],
                    ins=[hpre_loc.opt()], outs=[hpre_full.opt()])

            # ---- message passing: gather + one-hot matmul aggregation
            def message_pass(h_full, agg_sb):
                if skip_mp:
                    return
                tiles = {}     # slice idx -> (m_tile, dl_tile)

                def ensure(si):
                    if si in tiles or si >= len(SLICES):
                        return
                    w_, start, ln = SLICES[si]
                    src_hi = min((w_ + 1) * WIN, NCORES * npp)
                    src_win = h_full[w_ * WIN:src_hi, :]
                    s16, c = ln // 16, ln // P
                    o16, oc = start // 16, start // P
                    git = io.tile([P, s16], i16, tag="git")
                    nc.sync.dma_start(git[:], gidx_d[:, o16:o16 + s16])
                    dlt = io.tile([P, c], f32, tag="dlt")
                    nc.sync.dma_start(dlt[:], dloc_d[:, oc:oc + c])
                    ewt = io.tile([P, c, 1], f32, tag="ewt")
                    nc.sync.dma_start(ewt[:, :, 0], ew_d[:, oc:oc + c])
                    g = gm.tile([P, c, HID], f32, tag="g")
                    nc.gpsimd.dma_gather(
                        out_ap=g[:], in_ap=src_win, idxs_ap=git[:],
                        num_idxs=ln, num_idxs_reg=ln, elem_size=HID,
                        single_packet=False, queue_num=gq[si % len(gq)])
                    m = gm.tile([P, c, HID], f32, tag="m")
                    nc.vector.tensor_tensor(
                        out=m[:], in0=g[:],
                        in1=ewt[:].to_broadcast([P, c, HID]),
                        op=mybir.AluOpType.mult)
                    tiles[si] = (m, dlt)

                for (t_, k0, nch) in GROUPS:
                    ps = psAgg.tile([P, HID], f32, tag="agg")
                    for j in range(nch):
                        k = k0 + j
                        si = chunk2slice[k]
                        ensure(si)
                        ensure(si + 1)
                        m, dlt = tiles[si]
                        col = k - slice_chunk0[si]
                        oh = ohp.tile([P, P], f32, tag="oh")
                        nc.vector.tensor_tensor(
                            out=oh[:],
                            in0=dlt[:, col:col + 1].to_broadcast([P, P]),
                            in1=iota_row[:],
                            op=mybir.AluOpType.is_equal)
                        nc.tensor.matmul(ps[:], lhsT=oh[:], rhs=m[:, col, :],
                                         start=(j == 0), stop=(j == nch - 1))
                    a = agg_sb[:, t_ * HID:(t_ + 1) * HID]
                    nc.vector.tensor_tensor(out=a, in0=a, in1=ps[:],
                                            op=mybir.AluOpType.add)

            message_pass(hpre_full, agg1_sb)

            # ---- phase D: h1 = relu(agg1+b1); h2_pre = h1 @ W2
            for t in range(NT):
                tp = psT.tile([HID, P], f32, tag="tp")
                nc.tensor.transpose(out=tp[:],
                                    in_=agg1_sb[:, t * HID:(t + 1) * HID],
                                    identity=ident[:])
                h1t = hsb.tile([HID, P], f32, tag="h1t")
                nc.scalar.activation(h1t[:], tp[:],
                                     mybir.ActivationFunctionType.Relu,
                                     bias=b1_s[:], scale=1.0)
                ps2 = psB.tile([P, HID], f32, tag="psB")
                nc.tensor.matmul(ps2[:], lhsT=h1t[:], rhs=W2_s[:],
                                 start=True, stop=True)
                hb2 = hsb.tile([P, HID], f32, tag="hb2")
                nc.vector.tensor_copy(hb2[:], ps2[:])
                nc.sync.dma_start(h2pre_loc[t * P:(t + 1) * P, :], hb2[:])

            if not skip_ag:
                nc.gpsimd.collective_compute(
                    "AllGather", mybir.AluOpType.bypass,
                    replica_groups=[list(range(NCORES))],
                    ins=[h2pre_loc.opt()], outs=[h2pre_full.opt()])

            message_pass(h2pre_full, agg2_sb)

            # ---- phase G: h2 = relu(agg2+b2); pooled += h2.T @ pool_onehot
            pool_ps = psPool.tile([HID, G], f32)
            for t in range(NT):
                h2 = hsb.tile([P, HID], f32, tag="h2")
                nc.vector.tensor_tensor(out=h2[:],
                                        in0=agg2_sb[:, t * HID:(t + 1) * HID],
                                        in1=b2_s[:], op=mybir.AluOpType.add)
                nc.vector.tensor_scalar(out=h2[:], in0=h2[:], scalar1=0.0,
                                        scalar2=None, op0=mybir.AluOpType.max)
                pt = hsb.tile([P, G], f32, tag="pt")
                nc.sync.dma_start(pt[:], pool_d[t * P:(t + 1) * P, :])
                nc.tensor.matmul(pool_ps[:], lhsT=h2[:], rhs=pt[:],
                                 start=(t == 0), stop=(t == NT - 1))

            pool_sb = const.tile([HID, G], f32)
            nc.vector.tensor_copy(pool_sb[:], pool_ps[:])
            pool_in = dram.tile([HID, G], f32)
            pool_out = dram.tile([HID, G], f32)
            nc.sync.dma_start(pool_in[:], pool_sb[:])
            nc.gpsimd.collective_compute(
                "AllReduce", mybir.AluOpType.add,
                replica_groups=[list(range(NCORES))],
                ins=[pool_in.opt()], outs=[pool_out.opt()])
            nc.sync.dma_start(out_d[:], pool_out[:])

    nc.compile()
    return nc


# ------------------------------------------------------------------ runner

def run_spmd(nc, in_maps, n_cores=8, time_runs=0):
    """run_bass_via_pjrt clone that can re-execute the jitted NEFF for timing.

    Returns (results, exec_seconds|None): results from the first execution,
    exec_seconds = best wall-clock of `time_runs` repeat executions with
    device-resident, properly sharded inputs.
    """
    import time as _time
    import jax
    import jax.numpy as jnp
    from jax.sharding import Mesh, PartitionSpec
    from jax.experimental.shard_map import shard_map
    from concourse import bass2jax
    import concourse.mybir as mybir

    bass2jax.install_neuronx_cc_hook()
    part_name = nc.partition_id_tensor.name if nc.partition_id_tensor else None
    in_names, out_names, out_avals, zero_shapes = [], [], [], []
    for alloc in nc.m.functions[0].allocations:
        if not isinstance(alloc, mybir.MemoryLocationSet):
            continue
        name = alloc.memorylocations[0].name
        if alloc.kind == "ExternalInput":
            if name != part_name:
                in_names.append(name)
        elif alloc.kind == "ExternalOutput":
            out_names.append(name)
            shape = tuple(alloc.tensor_shape)
            dtype = mybir.dt.np(alloc.dtype)
            out_avals.append(jax.core.ShapedArray(shape, dtype))
            zero_shapes.append((shape, dtype))
    n_params = len(in_names)
    all_names = in_names + out_names
    if part_name is not None:
        all_names = all_names + [part_name]

    def _body(*args):
        operands = list(args)
        if part_name is not None:
            operands.append(bass2jax.partition_id_tensor())
        return tuple(bass2jax._bass_exec_p.bind(
            *operands, out_avals=tuple(out_avals), in_names=tuple(all_names),
            out_names=tuple(out_names), lowering_input_output_aliases=(),
            sim_require_finite=True, sim_require_nnan=True, nc=nc))

    devices = jax.devices()[:n_cores]
    mesh = Mesh(np.asarray(devices), ("core",))
    donate = tuple(range(n_params, n_params + len(out_names)))
    sharded = jax.jit(
        shard_map(_body, mesh=mesh,
                  in_specs=(PartitionSpec("core"),) * (n_params + len(out_names)),
                  out_specs=(PartitionSpec("core"),) * len(out_names),
                  check_rep=False),
        donate_argnums=donate, keep_unused=True)

    concat_in = [np.concatenate([np.asarray(in_maps[c][n]) for c in range(n_cores)], 0)
                 for n in in_names]
    def zeros():
        return [np.zeros((n_cores * s[0], *s[1:]), d) for s, d in zero_shapes]

    out_arrs = sharded(*concat_in, *zeros())
    jax.block_until_ready(out_arrs)
    results = [
        {n: np.asarray(out_arrs[i]).reshape(n_cores, *out_avals[i].shape)[c]
         for i, n in enumerate(out_names)}
        for c in range(n_cores)
    ]

    best = None
    if time_runs:
        from jax.sharding import NamedSharding
        shard = NamedSharding(mesh, PartitionSpec("core"))
        dev_in = [jax.device_put(jnp.asarray(a), shard) for a in concat_in]
        jax.block_until_ready(dev_in)
        for _ in range(time_runs):
            z = [jax.device_put(a, shard) for a in zeros()]
            jax.block_until_ready(z)
            t0 = _time.perf_counter()
            o = sharded(*dev_in, *z)
            jax.block_until_ready(o)
            dt = _time.perf_counter() - t0
            best = dt if best is None else min(best, dt)
    return results, best


_LAST_EXEC_S = None


def kernel(x, edge_index, edge_weight, batch, num_graphs, W1, b1, W2, b2):
    global _LAST_EXEC_S
    W1 = np.ascontiguousarray(np.asarray(W1, np.float32))
    b1 = np.ascontiguousarray(np.asarray(b1, np.float32)).reshape(-1, 1)
    W2 = np.ascontiguousarray(np.asarray(W2, np.float32))
    b2v = np.ascontiguousarray(np.asarray(b2, np.float32)).reshape(1, -1)
    b2r = np.tile(b2v, (P, 1))

    in_maps, meta = prep_host(x, edge_index, edge_weight, batch, num_graphs)
    for m in in_maps:
        m["W1"], m["b1"], m["W2"], m["b2"] = W1, b1, W2, b2r

    nc = build_program(meta, W1, b1, W2, b2r)
    import os
    time_runs = int(os.environ.get("GCN_TIME_RUNS", "25"))
    results, best = run_spmd(nc, in_maps, meta["n_cores"], time_runs=time_runs)
    _LAST_EXEC_S = best
    return np.ascontiguousarray(results[0]["out"].T.astype(np.float32))


# revision 22
# speedup vs baseline: 1.3386x; 1.0501x over previous
"""GCN encoder (2x GCNConv + global mean pool) on 8 Trainium2 NeuronCores.

Sharding: nodes are dst-sharded across 8 cores (12500 nodes each, padded to
12544).  Each core computes h_pre = x_c @ W for its slice and the slices are
AllGathered.  Message passing is gather-only: edges are grouped by
(src window, dst tile) with each group padded to a multiple of 128;
dma_gather (4 SWDGE queues, round-robin, so descriptor generation runs on
all 4 Q7 core pairs) pulls h_pre[src] in 4096-edge chunks with int16
window-local indices, DVE scales by edge weight, and aggregation runs on
the TensorEngine as one-hot matmuls accumulated in PSUM — a 128-edge
chunk's [128e x 128d] one-hot is built by DVE is_equal of the chunk's
dst-local ids against a host-provided iota row.  Per-tile results accumulate
into an SBUF buffer; there is no dma_scatter_add and no HBM read-modify-
write.  Bias+relu and the next-layer matmul read the SBUF accumulator per
128-node tile; global mean pool is a one-hot matmul with host-precomputed
1/count weights, finished with a tiny AllReduce so every core holds the
full [64,64] output.
"""
import sys
sys.path.insert(0, "/opt/trn_rl_repo")

import numpy as np

P = 128
GCHUNK = 4096         # edges per gather instruction
DMA_SCRATCH = 65536   # SWDGE descriptor carveout bytes/partition


def _wrap16(a):
    return a.reshape(-1, 16).T.copy()


def _wrap128(a):
    return a.reshape(-1, P).T.copy()


# ---------------------------------------------------------------- host prep

def prep_host(x, edge_index, edge_weight, batch, num_graphs, gchunk=GCHUNK):
    """Shard + pack inputs. Returns (in_maps, meta)."""
    x = np.ascontiguousarray(np.asarray(x, dtype=np.float32))
    src = np.asarray(edge_index[0], dtype=np.int64)
    dst = np.asarray(edge_index[1], dtype=np.int64)
    ew = np.asarray(edge_weight, dtype=np.float32)
    batch = np.asarray(batch, dtype=np.int64)
    G = int(num_graphs)

    n_cores = 8
    N, IN_CH = x.shape
    E = src.shape[0]
    npc = N // n_cores                      # real nodes per core
    npp = ((npc + P) // P) * P              # padded nodes per core
    nt = npp // P                           # dst tiles per core
    cpw = max(1, 32768 // npp)              # cores per gather window
    n_win = (n_cores + cpw - 1) // cpw
    win = cpw * npp                         # padded rows per window

    core = dst // npc
    tl = (dst % npc) // P                   # dst tile within core
    dl = ((dst % npc) % P).astype(np.float32)   # dst row within tile
    psrc = (src // npc) * npp + (src % npc)     # src id in padded node space
    w = psrc // win
    g16 = (psrc % win).astype(np.int16)     # window-local src id

    # shared (max-over-cores) 128-padded group lengths for (window, tile)
    nwt = n_win * nt
    key = core * nwt + w * nt + tl
    cnt = np.bincount(key, minlength=n_cores * nwt).reshape(n_cores, n_win, nt)
    cmax = cnt.max(axis=0)
    pl = ((cmax + P - 1) // P) * P
    pl[cmax == 0] = 0                       # [n_win, nt]
    flat = pl.ravel()
    off = np.concatenate([[0], np.cumsum(flat)])[:-1].reshape(n_win, nt)
    TOT = int(flat.sum())

    # per-edge stream position: group offset + within-(core,w,t) rank
    order = np.argsort(key, kind="stable")
    ks = key[order]
    new_g = np.r_[True, ks[1:] != ks[:-1]]
    gstart = np.maximum.accumulate(np.where(new_g, np.arange(E), 0))
    within = np.arange(E) - gstart
    pos = off[w[order], tl[order]] + within
    core_s = core[order]

    cnt_b = np.bincount(batch, minlength=G).astype(np.float32)
    inv_cnt = 1.0 / np.maximum(cnt_b, 1.0)

    g16_s, dl_s, ew_s = g16[order], dl[order], ew[order]
    in_maps = []
    for c in range(n_cores):
        sel = core_s == c
        p = pos[sel]
        gi = np.zeros(TOT, np.int16)          # pads gather row 0
        dli = np.full(TOT, -1.0, np.float32)  # pads match no dst row
        ewi = np.zeros(TOT, np.float32)       # pads contribute 0
        gi[p] = g16_s[sel]
        dli[p] = dl_s[sel]
        ewi[p] = ew_s[sel]

        xT = np.zeros((IN_CH, npp), np.float32)
        xT[:, :npc] = x[c * npc:(c + 1) * npc].T
        bc = batch[c * npc:(c + 1) * npc]
        pool = np.zeros((npp, G), np.float32)
        pool[np.arange(npc), bc] = inv_cnt[bc]

        in_maps.append({
            "xT": xT,
            "iota": np.tile(np.arange(P, dtype=np.float32), (P, 1)),
            "gidx": np.tile(_wrap16(gi), (8, 1)),
            "dloc": _wrap128(dli),
            "ew": _wrap128(ewi),
            "pool": pool,
        })

    # gather slices (w, stream start, len), each within one window
    slices = []
    for w_ in range(n_win):
        s = int(off[w_, 0])
        L = int(pl[w_].sum())
        p0 = 0
        while p0 < L:
            ln = min(gchunk, L - p0)
            slices.append((w_, s + p0, ln))
            p0 += ln

    # aggregation groups (tile, first chunk, n chunks) in stream order
    groups = []
    for w_ in range(n_win):
        for t_ in range(nt):
            L = int(pl[w_, t_])
            if L:
                groups.append((t_, int(off[w_, t_]) // P, L // P))

    meta = dict(n_cores=n_cores, N=N, IN_CH=IN_CH, npc=npc, npp=npp,
                n_tiles=nt, n_win=n_win, win=win, slices=slices,
                groups=groups, tot=TOT, G=G)
    return in_maps, meta


# ------------------------------------------------------------ program build

def build_program(meta, W1, b1, W2, b2, scratch=DMA_SCRATCH, gq=(0, 1, 2, 3),
                  skip_mp=False, skip_ag=False, ps_bufs=(2, 2, 1, 2),
                  batch_loads=1, inplace_m=False, use_bf16=False, gm_bufs=3,
                  oh_bufs=4, prefetch=1):
    import concourse.bass as bass
    import concourse.bacc as bacc
    import concourse.tile as tile
    import concourse.mybir as mybir
    from concourse.masks import make_identity

    f32 = mybir.dt.float32
    i16 = mybir.dt.int16
    fmsg = mybir.dt.bfloat16 if use_bf16 else f32
    NCORES = meta["n_cores"]
    IN_CH, npp = meta["IN_CH"], meta["npp"]
    NT, NW, WIN = meta["n_tiles"], meta["n_win"], meta["win"]
    SLICES, GROUPS, TOT, G = meta["slices"], meta["groups"], meta["tot"], meta["G"]
    HID = W1.shape[1]

    n_queues = max(gq) + 1
    nc = bacc.Bacc("TRN2", target_bir_lowering=False, debug=False,
                   num_devices=NCORES, dynamic_dma_scratch_size=scratch,
                   num_swdge_queues=n_queues)

    xT_d = nc.dram_tensor("xT", [IN_CH, npp], f32, kind="ExternalInput")
    iota_d = nc.dram_tensor("iota", [P, P], f32, kind="ExternalInput")
    gidx_d = nc.dram_tensor("gidx", [P, TOT // 16], i16, kind="ExternalInput")
    dloc_d = nc.dram_tensor("dloc", [P, TOT // P], f32, kind="ExternalInput")
    ew_d = nc.dram_tensor("ew", [P, TOT // P], f32, kind="ExternalInput")
    pool_d = nc.dram_tensor("pool", [npp, G], f32, kind="ExternalInput")
    W1_d = nc.dram_tensor("W1", [IN_CH, HID], f32, kind="ExternalInput")
    b1_d = nc.dram_tensor("b1", [HID, 1], f32, kind="ExternalInput")
    W2_d = nc.dram_tensor("W2", [HID, HID], f32, kind="ExternalInput")
    b2_d = nc.dram_tensor("b2", [P, HID], f32, kind="ExternalInput")
    out_d = nc.dram_tensor("out", [HID, G], f32, kind="ExternalOutput")

    # chunk index -> slice index
    chunk2slice = []
    slice_chunk0 = []
    for si, (w_, s, ln) in enumerate(SLICES):
        slice_chunk0.append(s // P)
        chunk2slice.extend([si] * (ln // P))

    with tile.TileContext(nc) as tc:
        with (
            tc.tile_pool(name="const", bufs=1) as const,
            tc.tile_pool(name="dram", bufs=1, space="DRAM") as dram,
            tc.tile_pool(name="io", bufs=3) as io,
            tc.tile_pool(name="gm", bufs=gm_bufs) as gm,
            tc.tile_pool(name="oh", bufs=oh_bufs) as ohp,
            tc.tile_pool(name="hsb", bufs=3) as hsb,
            tc.tile_pool(name="psA", bufs=ps_bufs[0], space="PSUM") as psA,
            tc.tile_pool(name="psAgg", bufs=ps_bufs[1], space="PSUM") as psAgg,
            tc.tile_pool(name="psT", bufs=ps_bufs[2], space="PSUM") as psT,
            tc.tile_pool(name="psB", bufs=ps_bufs[3], space="PSUM") as psB,
            tc.tile_pool(name="psPool", bufs=1, space="PSUM") as psPool,
        ):
            W1_s = const.tile([IN_CH, HID], f32)
            nc.sync.dma_start(W1_s[:], W1_d[:])
            W2_s = const.tile([HID, HID], f32)
            nc.sync.dma_start(W2_s[:], W2_d[:])
            b1_s = const.tile([HID, 1], f32)
            nc.sync.dma_start(b1_s[:], b1_d[:])
            b2_s = const.tile([P, HID], f32)
            nc.sync.dma_start(b2_s[:], b2_d[:])
            ident = const.tile([P, P], f32)
            make_identity(nc, ident[:])
            # iota_row[p, j] = j  (host-provided f32 constant)
            iota_row = const.tile([P, P], f32)
            nc.sync.dma_start(iota_row[:], iota_d[:])

            hpre_loc = dram.tile([npp, HID], f32)
            hpre_full = dram.tile([NCORES * npp, HID], f32)
            h2pre_loc = dram.tile([npp, HID], f32)
            h2pre_full = dram.tile([NCORES * npp, HID], f32)

            agg1_sb = const.tile([P, NT * HID], f32)
            nc.gpsimd.memset(agg1_sb[:], 0.0)
            agg2_sb = const.tile([P, NT * HID], f32)
            nc.gpsimd.memset(agg2_sb[:], 0.0)

            # ---- phase A: h1_pre = x @ W1 (node-major via xT as lhsT)
            for t in range(NT):
                xs = io.tile([IN_CH, P], f32, tag="xs")
                nc.sync.dma_start(xs[:], xT_d[:, t * P:(t + 1) * P])
                ps = psA.tile([P, HID], f32, tag="psA")
                nc.tensor.matmul(ps[:], lhsT=xs[:], rhs=W1_s[:],
                                 start=True, stop=True)
                hb = hsb.tile([P, HID], f32, tag="hb")
                nc.vector.tensor_copy(hb[:], ps[:])
                nc.sync.dma_start(hpre_loc[t * P:(t + 1) * P, :], hb[:])

            if not skip_ag:
                nc.gpsimd.collective_compute(
                    "AllGather", mybir.AluOpType.bypass,
                    replica_groups=[list(range(NCORES))],
                    ins=[hpre_loc.opt()], outs=[hpre_full.opt()])

            # ---- message passing: gather + one-hot matmul aggregation
            def message_pass(h_full, agg_sb):
                if skip_mp:
                    return
                tiles = {}     # slice idx -> (m_tile, dl_tile)
                batches = {}   # batch idx -> (git, dlt, ewt, batch start)
                B = batch_loads

                def ensure_batch(b):
                    if b in batches:
                        return batches[b]
                    lo = B * b
                    hi = min(B * b + B, len(SLICES))
                    start = SLICES[lo][1]
                    ln = SLICES[hi - 1][1] + SLICES[hi - 1][2] - start
                    s16, c = ln // 16, ln // P
                    o16, oc = start // 16, start // P
                    git = io.tile([P, GCHUNK * B // 16], i16, tag="git")
                    nc.sync.dma_start(git[:, :s16], gidx_d[:, o16:o16 + s16])
                    dlt = io.tile([P, GCHUNK * B // P], f32, tag="dlt")
                    nc.sync.dma_start(dlt[:, :c], dloc_d[:, oc:oc + c])
                    ewt = io.tile([P, GCHUNK * B // P, 1], f32, tag="ewt")
                    nc.sync.dma_start(ewt[:, :c, 0], ew_d[:, oc:oc + c])
                    batches[b] = (git, dlt, ewt, start)
                    return batches[b]

                def ensure(si):
                    if si in tiles or si >= len(SLICES):
                        return
                    w_, start, ln = SLICES[si]
                    src_hi = min((w_ + 1) * WIN, NCORES * npp)
                    src_win = h_full[w_ * WIN:src_hi, :]
                    s16, c = ln // 16, ln // P
                    git, dltB, ewtB, bstart = ensure_batch(si // B)
                    o16, oc = (start - bstart) // 16, (start - bstart) // P
                    g = gm.tile([P, c, HID], f32, tag="g")
                    nc.gpsimd.dma_gather(
                        out_ap=g[:], in_ap=src_win,
                        idxs_ap=git[:, o16:o16 + s16],
                        num_idxs=ln, num_idxs_reg=ln, elem_size=HID,
                        single_packet=False, queue_num=gq[si % len(gq)])
                    if inplace_m:
                        m = g
                    else:
                        m = gm.tile([P, c, HID], fmsg, tag="m")
                    nc.vector.tensor_tensor(
                        out=m[:], in0=g[:],
                        in1=ewtB[:, oc:oc + c, :].to_broadcast([P, c, HID]),
                        op=mybir.AluOpType.mult)
                    tiles[si] = (m, dltB[:, oc:oc + c])

                for (t_, k0, nch) in GROUPS:
                    ps = psAgg.tile([P, HID], f32, tag="agg")
                    for j in range(nch):
                        k = k0 + j
                        si = chunk2slice[k]
                        for pf in range(prefetch + 1):
                            ensure(si + pf)
                        m, dlt = tiles[si]
                        col = k - slice_chunk0[si]
                        oh = ohp.tile([P, P], fmsg, tag="oh")
                        nc.vector.tensor_tensor(
                            out=oh[:],
                            in0=dlt[:, col:col + 1].to_broadcast([P, P]),
                            in1=iota_row[:],
                            op=mybir.AluOpType.is_equal)
                        nc.tensor.matmul(ps[:], lhsT=oh[:], rhs=m[:, col, :],
                                         start=(j == 0), stop=(j == nch - 1))
                    a = agg_sb[:, t_ * HID:(t_ + 1) * HID]
                    nc.vector.tensor_tensor(out=a, in0=a, in1=ps[:],
                                            op=mybir.AluOpType.add)

            message_pass(hpre_full, agg1_sb)

            # ---- phase D: h1 = relu(agg1+b1); h2_pre = h1 @ W2
            for t in range(NT):
                tp = psT.tile([HID, P], f32, tag="tp")
                nc.tensor.transpose(out=tp[:],
                                    in_=agg1_sb[:, t * HID:(t + 1) * HID],
                                    identity=ident[:])
                h1t = hsb.tile([HID, P], f32, tag="h1t")
                nc.scalar.activation(h1t[:], tp[:],
                                     mybir.ActivationFunctionType.Relu,
                                     bias=b1_s[:], scale=1.0)
                ps2 = psB.tile([P, HID], f32, tag="psB")
                nc.tensor.matmul(ps2[:], lhsT=h1t[:], rhs=W2_s[:],
                                 start=True, stop=True)
                hb2 = hsb.tile([P, HID], f32, tag="hb2")
                nc.vector.tensor_copy(hb2[:], ps2[:])
                nc.sync.dma_start(h2pre_loc[t * P:(t + 1) * P, :], hb2[:])

            if not skip_ag:
                nc.gpsimd.collective_compute(
                    "AllGather", mybir.AluOpType.bypass,
                    replica_groups=[list(range(NCORES))],
                    ins=[h2pre_loc.opt()], outs=[h2pre_full.opt()])

            message_pass(h2pre_full, agg2_sb)

            # ---- phase G: h2 = relu(agg2+b2); pooled += h2.T @ pool_onehot
            pool_ps = psPool.tile([HID, G], f32)
            for t in range(NT):
                h2 = hsb.tile([P, HID], f32, tag="h2")
                nc.vector.tensor_tensor(out=h2[:],
                                        in0=agg2_sb[:, t * HID:(t + 1) * HID],
                                        in1=b2_s[:], op=mybir.AluOpType.add)
                nc.vector.tensor_scalar(out=h2[:], in0=h2[:], scalar1=0.0,
                                        scalar2=None, op0=mybir.AluOpType.max)
                pt = hsb.tile([P, G], f32, tag="pt")
                nc.sync.dma_start(pt[:], pool_d[t * P:(t + 1) * P, :])
                nc.tensor.matmul(pool_ps[:], lhsT=h2[:], rhs=pt[:],
                                 start=(t == 0), stop=(t == NT - 1))

            pool_sb = const.tile([HID, G], f32)
            nc.vector.tensor_copy(pool_sb[:], pool_ps[:])
            pool_in = dram.tile([HID, G], f32)
            pool_out = dram.tile([HID, G], f32)
            nc.sync.dma_start(pool_in[:], pool_sb[:])
            nc.gpsimd.collective_compute(
                "AllReduce", mybir.AluOpType.add,
                replica_groups=[list(range(NCORES))],
                ins=[pool_in.opt()], outs=[pool_out.opt()])
            nc.sync.dma_start(out_d[:], pool_out[:])

    nc.compile()
    return nc


# ------------------------------------------------------------------ runner

def run_spmd(nc, in_maps, n_cores=8, time_runs=0):
    """run_bass_via_pjrt clone that can re-execute the jitted NEFF for timing.

    Returns (results, exec_seconds|None): results from the first execution,
    exec_seconds = best wall-clock of `time_runs` repeat executions with
    device-resident, properly sharded inputs.
    """
    import time as _time
    import jax
    import jax.numpy as jnp
    from jax.sharding import Mesh, PartitionSpec
    from jax.experimental.shard_map import shard_map
    from concourse import bass2jax
    import concourse.mybir as mybir

    bass2jax.install_neuronx_cc_hook()
    part_name = nc.partition_id_tensor.name if nc.partition_id_tensor else None
    in_names, out_names, out_avals, zero_shapes = [], [], [], []
    for alloc in nc.m.functions[0].allocations:
        if not isinstance(alloc, mybir.MemoryLocationSet):
            continue
        name = alloc.memorylocations[0].name
        if alloc.kind == "ExternalInput":
            if name != part_name:
                in_names.append(name)
        elif alloc.kind == "ExternalOutput":
            out_names.append(name)
            shape = tuple(alloc.tensor_shape)
            dtype = mybir.dt.np(alloc.dtype)
            out_avals.append(jax.core.ShapedArray(shape, dtype))
            zero_shapes.append((shape, dtype))
    n_params = len(in_names)
    all_names = in_names + out_names
    if part_name is not None:
        all_names = all_names + [part_name]

    def _body(*args):
        operands = list(args)
        if part_name is not None:
            operands.append(bass2jax.partition_id_tensor())
        return tuple(bass2jax._bass_exec_p.bind(
            *operands, out_avals=tuple(out_avals), in_names=tuple(all_names),
            out_names=tuple(out_names), lowering_input_output_aliases=(),
            sim_require_finite=True, sim_require_nnan=True, nc=nc))

    devices = jax.devices()[:n_cores]
    mesh = Mesh(np.asarray(devices), ("core",))
    donate = tuple(range(n_params, n_params + len(out_names)))
    sharded = jax.jit(
        shard_map(_body, mesh=mesh,
                  in_specs=(PartitionSpec("core"),) * (n_params + len(out_names)),
                  out_specs=(PartitionSpec("core"),) * len(out_names),
                  check_rep=False),
        donate_argnums=donate, keep_unused=True)

    concat_in = [np.concatenate([np.asarray(in_maps[c][n]) for c in range(n_cores)], 0)
                 for n in in_names]
    def zeros():
        return [np.zeros((n_cores * s[0], *s[1:]), d) for s, d in zero_shapes]

    out_arrs = sharded(*concat_in, *zeros())
    jax.block_until_ready(out_arrs)
    results = [
        {n: np.asarray(out_arrs[i]).reshape(n_cores, *out_avals[i].shape)[c]
         for i, n in enumerate(out_names)}
        for c in range(n_cores)
    ]

    best = None
    if time_runs:
        from jax.sharding import NamedSharding
        shard = NamedSharding(mesh, PartitionSpec("core"))
        dev_in = [jax.device_put(jnp.asarray(a), shard) for a in concat_in]
        jax.block_until_ready(dev_in)
        for i in range(time_runs):
            if i and time_runs >= 20 and i % 10 == 0:
                _time.sleep(2.0)   # span tunnel-latency regimes
            z = [jax.device_put(a, shard) for a in zeros()]
            jax.block_until_ready(z)
            t0 = _time.perf_counter()
            o = sharded(*dev_in, *z)
            jax.block_until_ready(o)
            dt = _time.perf_counter() - t0
            best = dt if best is None else min(best, dt)
    return results, best


_LAST_EXEC_S = None


def kernel(x, edge_index, edge_weight, batch, num_graphs, W1, b1, W2, b2):
    global _LAST_EXEC_S
    W1 = np.ascontiguousarray(np.asarray(W1, np.float32))
    b1 = np.ascontiguousarray(np.asarray(b1, np.float32)).reshape(-1, 1)
    W2 = np.ascontiguousarray(np.asarray(W2, np.float32))
    b2v = np.ascontiguousarray(np.asarray(b2, np.float32)).reshape(1, -1)
    b2r = np.tile(b2v, (P, 1))

    in_maps, meta = prep_host(x, edge_index, edge_weight, batch, num_graphs)
    for m in in_maps:
        m["W1"], m["b1"], m["W2"], m["b2"] = W1, b1, W2, b2r

    # bf16 one-hot/messages (PSUM accumulation stays fp32), 4-slice batched
    # index loads, 4-deep gather buffers.  2 SWDGE queues measured identical
    # to 4 for this gather-only pipeline and avoids the >2-queue instability
    # seen with the earlier gather+scatter kernel.
    nc = build_program(meta, W1, b1, W2, b2r, gq=(0, 1), use_bf16=True,
                       batch_loads=4, gm_bufs=4)
    import os
    time_runs = int(os.environ.get("GCN_TIME_RUNS", "25"))
    results, best = run_spmd(nc, in_maps, meta["n_cores"], time_runs=time_runs)
    _LAST_EXEC_S = best
    return np.ascontiguousarray(results[0]["out"].T.astype(np.float32))


# revision 27
# speedup vs baseline: 1.3390x; 1.0003x over previous
"""GCN encoder (2x GCNConv + global mean pool) on 8 Trainium2 NeuronCores.

Sharding: nodes are dst-sharded across 8 cores (12500 nodes each, padded to
12544).  Each core computes h_pre = x_c @ W for its slice and the slices are
AllGathered.  Message passing is gather-only: edges are grouped by
(src window, dst tile) with each group padded to a multiple of 128;
dma_gather (4 SWDGE queues, round-robin, so descriptor generation runs on
all 4 Q7 core pairs) pulls h_pre[src] in 4096-edge chunks with int16
window-local indices, DVE scales by edge weight, and aggregation runs on
the TensorEngine as one-hot matmuls accumulated in PSUM — a 128-edge
chunk's [128e x 128d] one-hot is built by DVE is_equal of the chunk's
dst-local ids against a host-provided iota row.  Per-tile results accumulate
into an SBUF buffer; there is no dma_scatter_add and no HBM read-modify-
write.  Bias+relu and the next-layer matmul read the SBUF accumulator per
128-node tile; global mean pool is a one-hot matmul with host-precomputed
1/count weights, finished with a tiny AllReduce so every core holds the
full [64,64] output.
"""
import sys
sys.path.insert(0, "/opt/trn_rl_repo")

import numpy as np

P = 128
GCHUNK = 4096         # edges per gather instruction
DMA_SCRATCH = 65536   # SWDGE descriptor carveout bytes/partition


def _wrap16(a):
    return a.reshape(-1, 16).T.copy()


def _wrap128(a):
    return a.reshape(-1, P).T.copy()


# ---------------------------------------------------------------- host prep

def prep_host(x, edge_index, edge_weight, batch, num_graphs, gchunk=GCHUNK,
              quarters=False):
    """Shard + pack inputs. Returns (in_maps, meta)."""
    x = np.ascontiguousarray(np.asarray(x, dtype=np.float32))
    src = np.asarray(edge_index[0], dtype=np.int64)
    dst = np.asarray(edge_index[1], dtype=np.int64)
    ew = np.asarray(edge_weight, dtype=np.float32)
    batch = np.asarray(batch, dtype=np.int64)
    G = int(num_graphs)

    n_cores = 8
    N, IN_CH = x.shape
    E = src.shape[0]
    npc = N // n_cores                      # real nodes per core
    npp = ((npc + P) // P) * P              # padded nodes per core
    nt = npp // P                           # dst tiles per core
    cpw = max(1, 32768 // npp)              # cores per gather window
    n_win = (n_cores + cpw - 1) // cpw
    win = cpw * npp                         # padded rows per window

    core = dst // npc
    tl = (dst % npc) // P                   # dst tile within core
    dl = ((dst % npc) % P).astype(np.float32)   # dst row within tile
    if quarters:
        # window w = quarter-slab w of every rank: h_full row layout is
        # w*win + rank*npq + (local row % npq), matching 4 chunked
        # AllGathers whose inputs are row-quarters of hpre_loc.
        npq = npp // n_win
        r = src // npc
        lrow = src % npc
        w = lrow // npq
        g16 = (r * npq + lrow % npq).astype(np.int16)
    else:
        psrc = (src // npc) * npp + (src % npc)  # src id in padded node space
        w = psrc // win
        g16 = (psrc % win).astype(np.int16)      # window-local src id

    # shared (max-over-cores) 128-padded group lengths for (window, tile)
    nwt = n_win * nt
    key = core * nwt + w * nt + tl
    cnt = np.bincount(key, minlength=n_cores * nwt).reshape(n_cores, n_win, nt)
    cmax = cnt.max(axis=0)
    pl = ((cmax + P - 1) // P) * P
    pl[cmax == 0] = 0                       # [n_win, nt]
    flat = pl.ravel()
    off = np.concatenate([[0], np.cumsum(flat)])[:-1].reshape(n_win, nt)
    TOT = int(flat.sum())

    # per-edge stream position: group offset + within-(core,w,t) rank
    order = np.argsort(key, kind="stable")
    ks = key[order]
    new_g = np.r_[True, ks[1:] != ks[:-1]]
    gstart = np.maximum.accumulate(np.where(new_g, np.arange(E), 0))
    within = np.arange(E) - gstart
    pos = off[w[order], tl[order]] + within
    core_s = core[order]

    cnt_b = np.bincount(batch, minlength=G).astype(np.float32)
    inv_cnt = 1.0 / np.maximum(cnt_b, 1.0)

    g16_s, dl_s, ew_s = g16[order], dl[order], ew[order]
    in_maps = []
    for c in range(n_cores):
        sel = core_s == c
        p = pos[sel]
        gi = np.zeros(TOT, np.int16)          # pads gather row 0
        dli = np.full(TOT, -1.0, np.float32)  # pads match no dst row
        ewi = np.zeros(TOT, np.float32)       # pads contribute 0
        gi[p] = g16_s[sel]
        dli[p] = dl_s[sel]
        ewi[p] = ew_s[sel]

        xT = np.zeros((IN_CH, npp), np.float32)
        xT[:, :npc] = x[c * npc:(c + 1) * npc].T
        bc = batch[c * npc:(c + 1) * npc]
        pool = np.zeros((npp, G), np.float32)
        pool[np.arange(npc), bc] = inv_cnt[bc]

        in_maps.append({
            "xT": xT,
            "iota": np.tile(np.arange(P, dtype=np.float32), (P, 1)),
            "gidx": np.tile(_wrap16(gi), (8, 1)),
            "dloc": _wrap128(dli),
            "ew": _wrap128(ewi),
            "pool": pool,
        })

    # gather slices (w, stream start, len), each within one window
    slices = []
    for w_ in range(n_win):
        s = int(off[w_, 0])
        L = int(pl[w_].sum())
        p0 = 0
        while p0 < L:
            ln = min(gchunk, L - p0)
            slices.append((w_, s + p0, ln))
            p0 += ln

    # aggregation groups (tile, first chunk, n chunks) in stream order
    groups = []
    for w_ in range(n_win):
        for t_ in range(nt):
            L = int(pl[w_, t_])
            if L:
                groups.append((t_, int(off[w_, t_]) // P, L // P))

    meta = dict(n_cores=n_cores, N=N, IN_CH=IN_CH, npc=npc, npp=npp,
                n_tiles=nt, n_win=n_win, win=win, slices=slices,
                groups=groups, tot=TOT, G=G)
    return in_maps, meta


# ------------------------------------------------------------ program build

def build_program(meta, W1, b1, W2, b2, scratch=DMA_SCRATCH, gq=(0, 1, 2, 3),
                  skip_mp=False, skip_ag=False, ps_bufs=(2, 2, 1, 2),
                  batch_loads=1, inplace_m=False, use_bf16=False, gm_bufs=3,
                  oh_bufs=4, prefetch=1, chunked_ag=False):
    import concourse.bass as bass
    import concourse.bacc as bacc
    import concourse.tile as tile
    import concourse.mybir as mybir
    from concourse.masks import make_identity

    f32 = mybir.dt.float32
    i16 = mybir.dt.int16
    fmsg = mybir.dt.bfloat16 if use_bf16 else f32
    NCORES = meta["n_cores"]
    IN_CH, npp = meta["IN_CH"], meta["npp"]
    NT, NW, WIN = meta["n_tiles"], meta["n_win"], meta["win"]
    SLICES, GROUPS, TOT, G = meta["slices"], meta["groups"], meta["tot"], meta["G"]
    HID = W1.shape[1]

    n_queues = max(gq) + 1
    nc = bacc.Bacc("TRN2", target_bir_lowering=False, debug=False,
                   num_devices=NCORES, dynamic_dma_scratch_size=scratch,
                   num_swdge_queues=n_queues)

    xT_d = nc.dram_tensor("xT", [IN_CH, npp], f32, kind="ExternalInput")
    iota_d = nc.dram_tensor("iota", [P, P], f32, kind="ExternalInput")
    gidx_d = nc.dram_tensor("gidx", [P, TOT // 16], i16, kind="ExternalInput")
    dloc_d = nc.dram_tensor("dloc", [P, TOT // P], f32, kind="ExternalInput")
    ew_d = nc.dram_tensor("ew", [P, TOT // P], f32, kind="ExternalInput")
    pool_d = nc.dram_tensor("pool", [npp, G], f32, kind="ExternalInput")
    W1_d = nc.dram_tensor("W1", [IN_CH, HID], f32, kind="ExternalInput")
    b1_d = nc.dram_tensor("b1", [HID, 1], f32, kind="ExternalInput")
    W2_d = nc.dram_tensor("W2", [HID, HID], f32, kind="ExternalInput")
    b2_d = nc.dram_tensor("b2", [P, HID], f32, kind="ExternalInput")
    out_d = nc.dram_tensor("out", [HID, G], f32, kind="ExternalOutput")

    # chunk index -> slice index
    chunk2slice = []
    slice_chunk0 = []
    for si, (w_, s, ln) in enumerate(SLICES):
        slice_chunk0.append(s // P)
        chunk2slice.extend([si] * (ln // P))

    with tile.TileContext(nc) as tc:
        with (
            tc.tile_pool(name="const", bufs=1) as const,
            tc.tile_pool(name="dram", bufs=1, space="DRAM") as dram,
            tc.tile_pool(name="io", bufs=3) as io,
            tc.tile_pool(name="gm", bufs=gm_bufs) as gm,
            tc.tile_pool(name="oh", bufs=oh_bufs) as ohp,
            tc.tile_pool(name="hsb", bufs=3) as hsb,
            tc.tile_pool(name="psA", bufs=ps_bufs[0], space="PSUM") as psA,
            tc.tile_pool(name="psAgg", bufs=ps_bufs[1], space="PSUM") as psAgg,
            tc.tile_pool(name="psT", bufs=ps_bufs[2], space="PSUM") as psT,
            tc.tile_pool(name="psB", bufs=ps_bufs[3], space="PSUM") as psB,
            tc.tile_pool(name="psPool", bufs=1, space="PSUM") as psPool,
        ):
            W1_s = const.tile([IN_CH, HID], f32)
            nc.sync.dma_start(W1_s[:], W1_d[:])
            W2_s = const.tile([HID, HID], f32)
            nc.sync.dma_start(W2_s[:], W2_d[:])
            b1_s = const.tile([HID, 1], f32)
            nc.sync.dma_start(b1_s[:], b1_d[:])
            b2_s = const.tile([P, HID], f32)
            nc.sync.dma_start(b2_s[:], b2_d[:])
            ident = const.tile([P, P], f32)
            make_identity(nc, ident[:])
            # iota_row[p, j] = j  (host-provided f32 constant)
            iota_row = const.tile([P, P], f32)
            nc.sync.dma_start(iota_row[:], iota_d[:])

            hpre_loc = dram.tile([npp, HID], f32)
            hpre_full = dram.tile([NCORES * npp, HID], f32)
            h2pre_loc = dram.tile([npp, HID], f32)
            h2pre_full = dram.tile([NCORES * npp, HID], f32)

            agg1_sb = const.tile([P, NT * HID], f32)
            nc.gpsimd.memset(agg1_sb[:], 0.0)
            agg2_sb = const.tile([P, NT * HID], f32)
            nc.gpsimd.memset(agg2_sb[:], 0.0)

            # ---- phase A: h1_pre = x @ W1 (node-major via xT as lhsT)
            for t in range(NT):
                xs = io.tile([IN_CH, P], f32, tag="xs")
                nc.sync.dma_start(xs[:], xT_d[:, t * P:(t + 1) * P])
                ps = psA.tile([P, HID], f32, tag="psA")
                nc.tensor.matmul(ps[:], lhsT=xs[:], rhs=W1_s[:],
                                 start=True, stop=True)
                hb = hsb.tile([P, HID], f32, tag="hb")
                nc.vector.tensor_copy(hb[:], ps[:])
                nc.sync.dma_start(hpre_loc[t * P:(t + 1) * P, :], hb[:])

            def all_gather(loc, full):
                if skip_ag:
                    return
                if not chunked_ag:
                    nc.gpsimd.collective_compute(
                        "AllGather", mybir.AluOpType.bypass,
                        replica_groups=[list(range(NCORES))],
                        ins=[loc.opt()], outs=[full.opt()])
                    return
                npq = npp // NW
                for q in range(NW):
                    nc.gpsimd.collective_compute(
                        "AllGather", mybir.AluOpType.bypass,
                        replica_groups=[list(range(NCORES))],
                        ins=[loc[q * npq:(q + 1) * npq, :].opt()],
                        outs=[full[q * WIN:(q + 1) * WIN, :].opt()])

            all_gather(hpre_loc, hpre_full)

            # ---- message passing: gather + one-hot matmul aggregation
            def message_pass(h_full, agg_sb):
                if skip_mp:
                    return
                tiles = {}     # slice idx -> (m_tile, dl_tile)
                batches = {}   # batch idx -> (git, dlt, ewt, batch start)
                B = batch_loads

                def ensure_batch(b):
                    if b in batches:
                        return batches[b]
                    lo = B * b
                    hi = min(B * b + B, len(SLICES))
                    start = SLICES[lo][1]
                    ln = SLICES[hi - 1][1] + SLICES[hi - 1][2] - start
                    s16, c = ln // 16, ln // P
                    o16, oc = start // 16, start // P
                    git = io.tile([P, GCHUNK * B // 16], i16, tag="git")
                    nc.sync.dma_start(git[:, :s16], gidx_d[:, o16:o16 + s16])
                    dlt = io.tile([P, GCHUNK * B // P], f32, tag="dlt")
                    nc.sync.dma_start(dlt[:, :c], dloc_d[:, oc:oc + c])
                    ewt = io.tile([P, GCHUNK * B // P, 1], f32, tag="ewt")
                    nc.sync.dma_start(ewt[:, :c, 0], ew_d[:, oc:oc + c])
                    batches[b] = (git, dlt, ewt, start)
                    return batches[b]

                def ensure(si):
                    if si in tiles or si >= len(SLICES):
                        return
                    w_, start, ln = SLICES[si]
                    src_hi = min((w_ + 1) * WIN, NCORES * npp)
                    src_win = h_full[w_ * WIN:src_hi, :]
                    s16, c = ln // 16, ln // P
                    git, dltB, ewtB, bstart = ensure_batch(si // B)
                    o16, oc = (start - bstart) // 16, (start - bstart) // P
                    g = gm.tile([P, c, HID], f32, tag="g")
                    nc.gpsimd.dma_gather(
                        out_ap=g[:], in_ap=src_win,
                        idxs_ap=git[:, o16:o16 + s16],
                        num_idxs=ln, num_idxs_reg=ln, elem_size=HID,
                        single_packet=False, queue_num=gq[si % len(gq)])
                    if inplace_m:
                        m = g
                    else:
                        m = gm.tile([P, c, HID], fmsg, tag="m")
                    nc.vector.tensor_tensor(
                        out=m[:], in0=g[:],
                        in1=ewtB[:, oc:oc + c, :].to_broadcast([P, c, HID]),
                        op=mybir.AluOpType.mult)
                    tiles[si] = (m, dltB[:, oc:oc + c])

                for (t_, k0, nch) in GROUPS:
                    ps = psAgg.tile([P, HID], f32, tag="agg")
                    for j in range(nch):
                        k = k0 + j
                        si = chunk2slice[k]
                        for pf in range(prefetch + 1):
                            ensure(si + pf)
                        m, dlt = tiles[si]
                        col = k - slice_chunk0[si]
                        oh = ohp.tile([P, P], fmsg, tag="oh")
                        nc.vector.tensor_tensor(
                            out=oh[:],
                            in0=dlt[:, col:col + 1].to_broadcast([P, P]),
                            in1=iota_row[:],
                            op=mybir.AluOpType.is_equal)
                        nc.tensor.matmul(ps[:], lhsT=oh[:], rhs=m[:, col, :],
                                         start=(j == 0), stop=(j == nch - 1))
                    a = agg_sb[:, t_ * HID:(t_ + 1) * HID]
                    nc.vector.tensor_tensor(out=a, in0=a, in1=ps[:],
                                            op=mybir.AluOpType.add)

            message_pass(hpre_full, agg1_sb)

            # ---- phase D: h1 = relu(agg1+b1); h2_pre = h1 @ W2
            for t in range(NT):
                tp = psT.tile([HID, P], f32, tag="tp")
                nc.tensor.transpose(out=tp[:],
                                    in_=agg1_sb[:, t * HID:(t + 1) * HID],
                                    identity=ident[:])
                h1t = hsb.tile([HID, P], f32, tag="h1t")
                nc.scalar.activation(h1t[:], tp[:],
                                     mybir.ActivationFunctionType.Relu,
                                     bias=b1_s[:], scale=1.0)
                ps2 = psB.tile([P, HID], f32, tag="psB")
                nc.tensor.matmul(ps2[:], lhsT=h1t[:], rhs=W2_s[:],
                                 start=True, stop=True)
                hb2 = hsb.tile([P, HID], f32, tag="hb2")
                nc.vector.tensor_copy(hb2[:], ps2[:])
                nc.sync.dma_start(h2pre_loc[t * P:(t + 1) * P, :], hb2[:])

            all_gather(h2pre_loc, h2pre_full)

            message_pass(h2pre_full, agg2_sb)

            # ---- phase G: h2 = relu(agg2+b2); pooled += h2.T @ pool_onehot
            pool_ps = psPool.tile([HID, G], f32)
            for t in range(NT):
                h2 = hsb.tile([P, HID], f32, tag="h2")
                nc.vector.tensor_tensor(out=h2[:],
                                        in0=agg2_sb[:, t * HID:(t + 1) * HID],
                                        in1=b2_s[:], op=mybir.AluOpType.add)
                nc.vector.tensor_scalar(out=h2[:], in0=h2[:], scalar1=0.0,
                                        scalar2=None, op0=mybir.AluOpType.max)
                pt = hsb.tile([P, G], f32, tag="pt")
                nc.sync.dma_start(pt[:], pool_d[t * P:(t + 1) * P, :])
                nc.tensor.matmul(pool_ps[:], lhsT=h2[:], rhs=pt[:],
                                 start=(t == 0), stop=(t == NT - 1))

            pool_sb = const.tile([HID, G], f32)
            nc.vector.tensor_copy(pool_sb[:], pool_ps[:])
            pool_in = dram.tile([HID, G], f32)
            pool_out = dram.tile([HID, G], f32)
            nc.sync.dma_start(pool_in[:], pool_sb[:])
            nc.gpsimd.collective_compute(
                "AllReduce", mybir.AluOpType.add,
                replica_groups=[list(range(NCORES))],
                ins=[pool_in.opt()], outs=[pool_out.opt()])
            nc.sync.dma_start(out_d[:], pool_out[:])

    nc.compile()
    return nc


# ------------------------------------------------------------------ runner

def run_spmd(nc, in_maps, n_cores=8, time_runs=0):
    """run_bass_via_pjrt clone that can re-execute the jitted NEFF for timing.

    Returns (results, exec_seconds|None): results from the first execution,
    exec_seconds = best wall-clock of `time_runs` repeat executions with
    device-resident, properly sharded inputs.
    """
    import time as _time
    import jax
    import jax.numpy as jnp
    from jax.sharding import Mesh, PartitionSpec
    from jax.experimental.shard_map import shard_map
    from concourse import bass2jax
    import concourse.mybir as mybir

    bass2jax.install_neuronx_cc_hook()
    part_name = nc.partition_id_tensor.name if nc.partition_id_tensor else None
    in_names, out_names, out_avals, zero_shapes = [], [], [], []
    for alloc in nc.m.functions[0].allocations:
        if not isinstance(alloc, mybir.MemoryLocationSet):
            continue
        name = alloc.memorylocations[0].name
        if alloc.kind == "ExternalInput":
            if name != part_name:
                in_names.append(name)
        elif alloc.kind == "ExternalOutput":
            out_names.append(name)
            shape = tuple(alloc.tensor_shape)
            dtype = mybir.dt.np(alloc.dtype)
            out_avals.append(jax.core.ShapedArray(shape, dtype))
            zero_shapes.append((shape, dtype))
    n_params = len(in_names)
    all_names = in_names + out_names
    if part_name is not None:
        all_names = all_names + [part_name]

    def _body(*args):
        operands = list(args)
        if part_name is not None:
            operands.append(bass2jax.partition_id_tensor())
        return tuple(bass2jax._bass_exec_p.bind(
            *operands, out_avals=tuple(out_avals), in_names=tuple(all_names),
            out_names=tuple(out_names), lowering_input_output_aliases=(),
            sim_require_finite=True, sim_require_nnan=True, nc=nc))

    devices = jax.devices()[:n_cores]
    mesh = Mesh(np.asarray(devices), ("core",))
    donate = tuple(range(n_params, n_params + len(out_names)))
    sharded = jax.jit(
        shard_map(_body, mesh=mesh,
                  in_specs=(PartitionSpec("core"),) * (n_params + len(out_names)),
                  out_specs=(PartitionSpec("core"),) * len(out_names),
                  check_rep=False),
        donate_argnums=donate, keep_unused=True)

    concat_in = [np.concatenate([np.asarray(in_maps[c][n]) for c in range(n_cores)], 0)
                 for n in in_names]
    def zeros():
        return [np.zeros((n_cores * s[0], *s[1:]), d) for s, d in zero_shapes]

    out_arrs = sharded(*concat_in, *zeros())
    jax.block_until_ready(out_arrs)
    results = [
        {n: np.asarray(out_arrs[i]).reshape(n_cores, *out_avals[i].shape)[c]
         for i, n in enumerate(out_names)}
        for c in range(n_cores)
    ]

    best = None
    if time_runs:
        from jax.sharding import NamedSharding
        shard = NamedSharding(mesh, PartitionSpec("core"))
        dev_in = [jax.device_put(jnp.asarray(a), shard) for a in concat_in]
        jax.block_until_ready(dev_in)
        for i in range(time_runs):
            if i and time_runs >= 20 and i % 10 == 0:
                _time.sleep(2.0)   # span tunnel-latency regimes
            z = [jax.device_put(a, shard) for a in zeros()]
            jax.block_until_ready(z)
            t0 = _time.perf_counter()
            o = sharded(*dev_in, *z)
            jax.block_until_ready(o)
            dt = _time.perf_counter() - t0
            best = dt if best is None else min(best, dt)
    return results, best


_LAST_EXEC_S = None


def kernel(x, edge_index, edge_weight, batch, num_graphs, W1, b1, W2, b2):
    global _LAST_EXEC_S
    W1 = np.ascontiguousarray(np.asarray(W1, np.float32))
    b1 = np.ascontiguousarray(np.asarray(b1, np.float32)).reshape(-1, 1)
    W2 = np.ascontiguousarray(np.asarray(W2, np.float32))
    b2v = np.ascontiguousarray(np.asarray(b2, np.float32)).reshape(1, -1)
    b2r = np.tile(b2v, (P, 1))

    in_maps, meta = prep_host(x, edge_index, edge_weight, batch, num_graphs)
    for m in in_maps:
        m["W1"], m["b1"], m["W2"], m["b2"] = W1, b1, W2, b2r

    # bf16 one-hot/messages (PSUM accumulation stays fp32), 4-slice batched
    # index loads, 4-deep gather buffers.  2 SWDGE queues measured identical
    # to 4 for this gather-only pipeline and avoids the >2-queue instability
    # seen with the earlier gather+scatter kernel.
    nc = build_program(meta, W1, b1, W2, b2r, gq=(0, 1), use_bf16=True,
                       batch_loads=4, gm_bufs=4)
    import os
    time_runs = int(os.environ.get("GCN_TIME_RUNS", "25"))
    results, best = run_spmd(nc, in_maps, meta["n_cores"], time_runs=time_runs)
    _LAST_EXEC_S = best
    return np.ascontiguousarray(results[0]["out"].T.astype(np.float32))
